# revision 9
# baseline (speedup 1.0000x reference)
"""Trainium2 Bass kernel for nn_Encoder_inter: coif1 wavelet disentangle along
the node axis (expressed as a dense 512x512 matrix, precomputed on host) followed
by a 2-layer MLP (64->256->256) with ReLU, pointwise over (B, N, T).

Sharding: data-parallel over batch B=32 across 8 NeuronCores (4 batches each);
the small Linear weights and the wavelet matrix are replicated.
"""
import os
import sys

for _p in ("/opt/trn_rl_repo", "/root/.axon_site/_ro/trn_rl_repo"):
    if os.path.isdir(_p) and _p not in sys.path:
        sys.path.insert(0, _p)

from contextlib import ExitStack

import numpy as np

import concourse.bass as bass
import concourse.tile as tile
from concourse import bacc, mybir
from concourse.bass_utils import run_bass_kernel_spmd

F32 = mybir.dt.float32
F32R = mybir.dt.float32r
BF16 = mybir.dt.bfloat16

# compute dtype for tensor-engine operands: "bf16" or "f32r"
COMPUTE = os.environ.get("KERNEL_COMPUTE_DTYPE", "bf16")
MM_DT = BF16 if COMPUTE == "bf16" else F32R

B, N, T, D, H, G = 32, 512, 24, 64, 256, 256
NCORES = 8
BPC = B // NCORES          # batches per core
TD = T * D                 # 1536
NCHUNK = N // 128          # 4
MCHUNK = N // 128          # 4
THALF = T // 2             # 12

# ---------------------------------------------------------------------------
# Host-side wavelet matrix: the whole dwt -> (2*cD) -> idwt chain along the
# node axis is linear, so it is exactly y = K @ x with K (N, N). We build
# K^T = op(eye(N)) in float64 with a numpy port of the reference transform.
# ---------------------------------------------------------------------------
_L = 6
_DEC_LO = np.array(
    [-0.01565572813546454, -0.0727326195128539, 0.38486484686420286,
     0.8525720202122554, 0.3378976624578092, -0.0727326195128539],
    dtype=np.float64,
)
_DEC_HI = np.array(
    [0.0727326195128539, 0.3378976624578092, -0.8525720202122554,
     0.38486484686420286, 0.0727326195128539, -0.01565572813546454],
    dtype=np.float64,
)
_REC_LO = _DEC_LO[::-1].copy()
_REC_HI = _DEC_HI[::-1].copy()


def _dwt_last(x):
    n = x.shape[-1]
    ext = np.concatenate(
        [x[..., : _L - 1][..., ::-1], x, x[..., -(_L - 1):][..., ::-1]], axis=-1
    )
    out = (n + _L - 2) // 2
    cA = sum(_DEC_LO[j] * ext[..., _L - j: _L - j + 2 * out: 2] for j in range(_L))
    cD = sum(_DEC_HI[j] * ext[..., _L - j: _L - j + 2 * out: 2] for j in range(_L))
    return cA, cD


def _idwt_last(cA, cD, n):
    out = cA.shape[-1]
    up_shape = cA.shape[:-1] + (2 * out - 1,)
    upA = np.zeros(up_shape, cA.dtype)
    upA[..., ::2] = cA
    upD = np.zeros(up_shape, cD.dtype)
    upD[..., ::2] = cD
    pad = [(0, 0)] * (cA.ndim - 1) + [(_L - 1, _L - 1)]
    uA = np.pad(upA, pad)
    uD = np.pad(upD, pad)
    return sum(
        _REC_LO[j] * uA[..., 2 * _L - 3 - j: 2 * _L - 3 - j + n]
        + _REC_HI[j] * uD[..., 2 * _L - 3 - j: 2 * _L - 3 - j + n]
        for j in range(_L)
    )


def _wavelet_kt() -> np.ndarray:
    """K^T (m_in, n_out) so that (op(x))[n] = sum_m x[m] * KT[m, n]."""
    eye = np.eye(N, dtype=np.float64)
    cA, cD = _dwt_last(eye)
    kt = _idwt_last(cA, 2.0 * cD, N)
    return kt.astype(np.float32)


# ---------------------------------------------------------------------------
# Device kernel (SPMD, identical program on all 8 cores)
# ---------------------------------------------------------------------------
_NC_CACHE = None


def _build_nc():
    nc = bacc.Bacc("TRN2", target_bir_lowering=False, debug=False, num_devices=NCORES)
    x_d = nc.dram_tensor("x", [BPC, MCHUNK, 128, TD], MM_DT, kind="ExternalInput").ap()
    kt_d = nc.dram_tensor("KT", [MCHUNK, 128, N], MM_DT, kind="ExternalInput").ap()
    w1_d = nc.dram_tensor("W1T", [D, H], MM_DT, kind="ExternalInput").ap()
    w2_d = nc.dram_tensor("W2T", [2, 128, G], MM_DT, kind="ExternalInput").ap()
    b1_d = nc.dram_tensor("b1", [2, 128, 1], F32, kind="ExternalInput").ap()
    b2f_d = nc.dram_tensor("b2f", [128, G], F32, kind="ExternalInput").ap()
    out_d = nc.dram_tensor("out", [BPC, N, T, G], F32, kind="ExternalOutput").ap()

    relu = mybir.ActivationFunctionType.Relu

    with tile.TileContext(nc) as tc, ExitStack() as ctx:
        consts = ctx.enter_context(tc.tile_pool(name="consts", bufs=1))
        xpool = ctx.enter_context(tc.tile_pool(name="xp", bufs=2))
        ypool = ctx.enter_context(tc.tile_pool(name="yp", bufs=3))
        hpool = ctx.enter_context(tc.tile_pool(name="hp", bufs=2))
        spool = ctx.enter_context(tc.tile_pool(name="sp", bufs=2))
        py = ctx.enter_context(tc.tile_pool(name="py", bufs=2, space="PSUM"))
        ph = ctx.enter_context(tc.tile_pool(name="ph", bufs=2, space="PSUM"))
        po = ctx.enter_context(tc.tile_pool(name="po", bufs=4, space="PSUM"))

        # --- replicated constants ---
        kt_sb = []
        for mc in range(MCHUNK):
            t_ = consts.tile([128, N], MM_DT, tag=f"kt{mc}", name=f"kt{mc}")
            nc.sync.dma_start(out=t_[:], in_=kt_d[mc])
            kt_sb.append(t_)
        w1_sb = consts.tile([D, H], MM_DT, tag="w1", name="w1")
        nc.sync.dma_start(out=w1_sb[:], in_=w1_d[:])
        w2_sb = []
        for hc in range(2):
            t_ = consts.tile([128, G], MM_DT, tag=f"w2{hc}", name=f"w2{hc}")
            nc.sync.dma_start(out=t_[:], in_=w2_d[hc])
            w2_sb.append(t_)
        b1_sb = []
        for hc in range(2):
            t_ = consts.tile([128, 1], F32, tag=f"b1{hc}", name=f"b1c{hc}")
            nc.sync.dma_start(out=t_[:], in_=b1_d[hc])
            b1_sb.append(t_)
        b2f_sb = consts.tile([128, G], F32, tag="b2f", name="b2f")
        nc.sync.dma_start(out=b2f_sb[:], in_=b2f_d[:])

        for b in range(BPC):
            x_sb = []
            for mc in range(MCHUNK):
                t_ = xpool.tile([128, TD], MM_DT, tag=f"x{mc}", name=f"xt{mc}")
                nc.sync.dma_start(out=t_[:], in_=x_d[b, mc])
                x_sb.append(t_)
            for half in range(2):
                stg = [
                    spool.tile([128, THALF * G], F32, tag=f"stg{nck}", name=f"stg{nck}")
                    for nck in range(NCHUNK)
                ]
                for tl in range(THALF):
                    t = half * THALF + tl
                    # step 1: y^T (d, n) = sum_m x[m, d] * KT[m, n]
                    yps = py.tile([D, N], F32, name="yps")
                    for mc in range(MCHUNK):
                        nc.tensor.matmul(
                            yps[:],
                            lhsT=x_sb[mc][:, t * D:(t + 1) * D],
                            rhs=kt_sb[mc][:],
                            start=(mc == 0),
                            stop=(mc == MCHUNK - 1),
                        )
                    y_sb = ypool.tile([D, N], MM_DT, tag="yt", name="y_sb")
                    nc.scalar.copy(y_sb[:], yps[:])
                    # step 2: h1^T (h, n) = relu(W1 @ y^T + b1)
                    h1 = []
                    for hc in range(2):
                        hps = ph.tile([128, N], F32, name="hps")
                        nc.tensor.matmul(
                            hps[:],
                            lhsT=w1_sb[:, hc * 128:(hc + 1) * 128],
                            rhs=y_sb[:],
                            start=True,
                            stop=True,
                        )
                        h_sb = hpool.tile([128, N], MM_DT, tag=f"h1_{hc}", name=f"h1_{hc}")
                        nc.scalar.activation(
                            h_sb[:], hps[:], relu, bias=b1_sb[hc][:], scale=1.0
                        )
                        h1.append(h_sb)
                    # step 3: out (n, g) = relu(h1 @ W2^T + b2)
                    for nck in range(NCHUNK):
                        ops = po.tile([128, G], F32, name="ops")
                        for hc in range(2):
                            nc.tensor.matmul(
                                ops[:],
                                lhsT=h1[hc][:, nck * 128:(nck + 1) * 128],
                                rhs=w2_sb[hc][:],
                                start=(hc == 0),
                                stop=(hc == 1),
                            )
                        dst = stg[nck][:, tl * G:(tl + 1) * G]
                        nc.vector.scalar_tensor_tensor(
                            out=dst,
                            in0=ops[:],
                            scalar=0.0,
                            in1=b2f_sb[:],
                            op0=mybir.AluOpType.bypass,
                            op1=mybir.AluOpType.add,
                        )
                        nc.gpsimd.tensor_scalar_max(dst, dst, 0.0)
                for nck in range(NCHUNK):
                    nc.sync.dma_start(
                        out=out_d[
                            b,
                            nck * 128:(nck + 1) * 128,
                            half * THALF:(half + 1) * THALF,
                            :,
                        ],
                        in_=stg[nck][:].rearrange("p (t g) -> p t g", t=THALF),
                    )
    nc.compile()
    return nc


def _get_nc():
    global _NC_CACHE
    if _NC_CACHE is None:
        _NC_CACHE = _build_nc()
    return _NC_CACHE


def _make_in_maps(x, W1, b1, W2, b2):
    if COMPUTE == "bf16":
        import ml_dtypes
        mmnp = ml_dtypes.bfloat16
    else:
        mmnp = np.float32
    x = np.ascontiguousarray(np.asarray(x, dtype=np.float32))
    W1 = np.asarray(W1, dtype=np.float32)
    b1 = np.asarray(b1, dtype=np.float32)
    W2 = np.asarray(W2, dtype=np.float32)
    b2 = np.asarray(b2, dtype=np.float32)

    kt = _wavelet_kt().reshape(MCHUNK, 128, N).astype(mmnp)
    w1t = np.ascontiguousarray(W1.T).astype(mmnp)              # (D, H)
    w2t = np.ascontiguousarray(W2.T).reshape(2, 128, G).astype(mmnp)
    b1r = np.ascontiguousarray(b1.reshape(2, 128, 1))
    b2f = np.ascontiguousarray(np.tile(b2.reshape(1, G), (128, 1)))

    in_maps = []
    for c in range(NCORES):
        xc = x[c * BPC:(c + 1) * BPC].reshape(BPC, N, TD)
        xc = np.ascontiguousarray(xc.reshape(BPC, MCHUNK, 128, TD).astype(mmnp))
        in_maps.append(
            {"x": xc, "KT": kt, "W1T": w1t, "W2T": w2t, "b1": b1r, "b2f": b2f}
        )
    return in_maps


def kernel(x, W1, b1, W2, b2):
    nc = _get_nc()
    in_maps = _make_in_maps(x, W1, b1, W2, b2)
    res = run_bass_kernel_spmd(nc, in_maps, list(range(NCORES)))
    out = np.concatenate([res.results[c]["out"] for c in range(NCORES)], axis=0)
    return out


# revision 11
# speedup vs baseline: 5.2598x; 5.2598x over previous
"""Trainium2 Bass kernel for nn_Encoder_inter: coif1 wavelet disentangle along
the node axis (expressed as a dense 512x512 matrix, precomputed on host) followed
by a 2-layer MLP (64->256->256) with ReLU, pointwise over (B, N, T).

Sharding: data-parallel over batch B=32 across 8 NeuronCores (4 batches each);
the small Linear weights and the wavelet matrix are replicated.
"""
import os
import sys

for _p in ("/opt/trn_rl_repo", "/root/.axon_site/_ro/trn_rl_repo"):
    if os.path.isdir(_p) and _p not in sys.path:
        sys.path.insert(0, _p)

from contextlib import ExitStack

import numpy as np

import concourse.bass as bass
import concourse.tile as tile
from concourse import bacc, mybir
from concourse.bass_utils import run_bass_kernel_spmd

F32 = mybir.dt.float32
F32R = mybir.dt.float32r
BF16 = mybir.dt.bfloat16

# compute dtype for tensor-engine operands: "bf16" or "f32r"
COMPUTE = os.environ.get("KERNEL_COMPUTE_DTYPE", "bf16")
MM_DT = BF16 if COMPUTE == "bf16" else F32R

B, N, T, D, H, G = 32, 512, 24, 64, 256, 256
NCORES = 8
BPC = B // NCORES          # batches per core
TD = T * D                 # 1536
NCHUNK = N // 128          # 4
MCHUNK = N // 128          # 4
THALF = T // 2             # 12

# ---------------------------------------------------------------------------
# Host-side wavelet matrix: the whole dwt -> (2*cD) -> idwt chain along the
# node axis is linear, so it is exactly y = K @ x with K (N, N). We build
# K^T = op(eye(N)) in float64 with a numpy port of the reference transform.
# ---------------------------------------------------------------------------
_L = 6
_DEC_LO = np.array(
    [-0.01565572813546454, -0.0727326195128539, 0.38486484686420286,
     0.8525720202122554, 0.3378976624578092, -0.0727326195128539],
    dtype=np.float64,
)
_DEC_HI = np.array(
    [0.0727326195128539, 0.3378976624578092, -0.8525720202122554,
     0.38486484686420286, 0.0727326195128539, -0.01565572813546454],
    dtype=np.float64,
)
_REC_LO = _DEC_LO[::-1].copy()
_REC_HI = _DEC_HI[::-1].copy()


def _dwt_last(x):
    n = x.shape[-1]
    ext = np.concatenate(
        [x[..., : _L - 1][..., ::-1], x, x[..., -(_L - 1):][..., ::-1]], axis=-1
    )
    out = (n + _L - 2) // 2
    cA = sum(_DEC_LO[j] * ext[..., _L - j: _L - j + 2 * out: 2] for j in range(_L))
    cD = sum(_DEC_HI[j] * ext[..., _L - j: _L - j + 2 * out: 2] for j in range(_L))
    return cA, cD


def _idwt_last(cA, cD, n):
    out = cA.shape[-1]
    up_shape = cA.shape[:-1] + (2 * out - 1,)
    upA = np.zeros(up_shape, cA.dtype)
    upA[..., ::2] = cA
    upD = np.zeros(up_shape, cD.dtype)
    upD[..., ::2] = cD
    pad = [(0, 0)] * (cA.ndim - 1) + [(_L - 1, _L - 1)]
    uA = np.pad(upA, pad)
    uD = np.pad(upD, pad)
    return sum(
        _REC_LO[j] * uA[..., 2 * _L - 3 - j: 2 * _L - 3 - j + n]
        + _REC_HI[j] * uD[..., 2 * _L - 3 - j: 2 * _L - 3 - j + n]
        for j in range(_L)
    )


def _wavelet_kt() -> np.ndarray:
    """K^T (m_in, n_out) so that (op(x))[n] = sum_m x[m] * KT[m, n]."""
    eye = np.eye(N, dtype=np.float64)
    cA, cD = _dwt_last(eye)
    kt = _idwt_last(cA, 2.0 * cD, N)
    return kt.astype(np.float32)


# ---------------------------------------------------------------------------
# Device kernel (SPMD, identical program on all 8 cores)
# ---------------------------------------------------------------------------
_NC_CACHE = None


def _build_nc():
    nc = bacc.Bacc("TRN2", target_bir_lowering=False, debug=False, num_devices=NCORES)
    x_d = nc.dram_tensor("x", [BPC, MCHUNK, 128, TD], MM_DT, kind="ExternalInput").ap()
    kt_d = nc.dram_tensor("KT", [MCHUNK, 128, N], MM_DT, kind="ExternalInput").ap()
    w1_d = nc.dram_tensor("W1T", [2 * D, H], MM_DT, kind="ExternalInput").ap()
    w2_d = nc.dram_tensor("W2T", [2, 128, G], MM_DT, kind="ExternalInput").ap()
    b1_d = nc.dram_tensor("b1", [2, 128, 1], F32, kind="ExternalInput").ap()
    b22_d = nc.dram_tensor("b22", [1, 2 * G], MM_DT, kind="ExternalInput").ap()
    ones_d = nc.dram_tensor("ones", [1, 128], MM_DT, kind="ExternalInput").ap()
    out_d = nc.dram_tensor("out", [BPC, N, T, G], F32, kind="ExternalOutput").ap()

    relu = mybir.ActivationFunctionType.Relu

    with tile.TileContext(nc) as tc, ExitStack() as ctx:
        consts = ctx.enter_context(tc.tile_pool(name="consts", bufs=1))
        xpool = ctx.enter_context(tc.tile_pool(name="xp", bufs=2))
        ypool = ctx.enter_context(tc.tile_pool(name="yp", bufs=3))
        hpool = ctx.enter_context(tc.tile_pool(name="hp", bufs=2))
        spool = ctx.enter_context(tc.tile_pool(name="sp", bufs=2))
        py = ctx.enter_context(tc.tile_pool(name="py", bufs=2, space="PSUM"))
        ph = ctx.enter_context(tc.tile_pool(name="ph", bufs=2, space="PSUM"))
        po = ctx.enter_context(tc.tile_pool(name="po", bufs=2, space="PSUM"))

        # --- replicated constants ---
        kt_sb = []
        for mc in range(MCHUNK):
            t_ = consts.tile([128, N], MM_DT, tag=f"kt{mc}", name=f"kt{mc}")
            nc.sync.dma_start(out=t_[:], in_=kt_d[mc])
            kt_sb.append(t_)
        w1_sb = consts.tile([2 * D, H], MM_DT, tag="w1", name="w1")
        nc.sync.dma_start(out=w1_sb[:], in_=w1_d[:])
        w2_sb = []
        for hc in range(2):
            t_ = consts.tile([128, G], MM_DT, tag=f"w2{hc}", name=f"w2{hc}")
            nc.sync.dma_start(out=t_[:], in_=w2_d[hc])
            w2_sb.append(t_)
        b1_sb = []
        for hc in range(2):
            t_ = consts.tile([128, 1], F32, tag=f"b1{hc}", name=f"b1c{hc}")
            nc.sync.dma_start(out=t_[:], in_=b1_d[hc])
            b1_sb.append(t_)
        b22_sb = consts.tile([1, 2 * G], MM_DT, tag="b22", name="b22")
        nc.sync.dma_start(out=b22_sb[:], in_=b22_d[:])
        ones_sb = consts.tile([1, 128], MM_DT, tag="ones", name="ones")
        nc.sync.dma_start(out=ones_sb[:], in_=ones_d[:])

        for b in range(BPC):
            x_sb = []
            for mc in range(MCHUNK):
                t_ = xpool.tile([128, TD], MM_DT, tag=f"x{mc}", name=f"xt{mc}")
                nc.sync.dma_start(out=t_[:], in_=x_d[b, mc])
                x_sb.append(t_)
            for half in range(2):
                stg = spool.tile(
                    [128, NCHUNK * THALF * G], F32, tag="stg", name="stg"
                )
                stg4 = stg[:].rearrange("p (k t g) -> p k t g", k=NCHUNK, t=THALF)
                for tp in range(THALF // 2):
                    t0 = half * THALF + 2 * tp
                    # step 1 (t-pair): psum rows = [t0 d | t1 d], cols = n
                    yps = py.tile([128, N], F32, name="yps")
                    for mc in range(MCHUNK):
                        nc.tensor.matmul(
                            yps[:],
                            lhsT=x_sb[mc][:, t0 * D:(t0 + 2) * D],
                            rhs=kt_sb[mc][:],
                            start=(mc == 0),
                            stop=(mc == MCHUNK - 1),
                        )
                    y_sb = ypool.tile([128, N], MM_DT, tag="yt", name="y_sb")
                    nc.scalar.copy(y_sb[:], yps[:])
                    # step 2: per hc, both t of the pair into one 2-bank psum
                    h1 = []
                    for hc in range(2):
                        hps = ph.tile([128, 2 * N], F32, name="hps")
                        for ti in range(2):
                            nc.tensor.matmul(
                                hps[:, ti * N:(ti + 1) * N],
                                lhsT=w1_sb[
                                    ti * D:(ti + 1) * D, hc * 128:(hc + 1) * 128
                                ],
                                rhs=y_sb[ti * D:(ti + 1) * D, :],
                                start=True,
                                stop=True,
                            )
                        h_sb = hpool.tile(
                            [128, 2 * N], MM_DT, tag=f"h1_{hc}", name=f"h1_{hc}"
                        )
                        nc.scalar.activation(
                            h_sb[:], hps[:], relu, bias=b1_sb[hc][:], scale=1.0
                        )
                        h1.append(h_sb)
                    # step 3: nck-pairs share one psum bank (128, 512)
                    for ti in range(2):
                        tl = 2 * tp + ti
                        for nckp in range(NCHUNK // 2):
                            ops = po.tile([128, 2 * G], F32, name="ops")
                            nc.tensor.matmul(
                                ops[:],
                                lhsT=ones_sb[:],
                                rhs=b22_sb[:],
                                start=True,
                                stop=False,
                                skip_group_check=True,
                            )
                            for sub in range(2):
                                nck = 2 * nckp + sub
                                for hc in range(2):
                                    nc.tensor.matmul(
                                        ops[:, sub * G:(sub + 1) * G],
                                        lhsT=h1[hc][
                                            :,
                                            ti * N + nck * 128:ti * N + (nck + 1) * 128,
                                        ],
                                        rhs=w2_sb[hc][:],
                                        start=False,
                                        stop=(sub == 1 and hc == 1),
                                        skip_group_check=True,
                                    )
                            nc.vector.tensor_scalar_max(
                                stg4[:, 2 * nckp:2 * nckp + 2, tl, :],
                                ops[:].rearrange("p (k g) -> p k g", k=2),
                                0.0,
                            )
                for nck in range(NCHUNK):
                    nc.sync.dma_start(
                        out=out_d[
                            b,
                            nck * 128:(nck + 1) * 128,
                            half * THALF:(half + 1) * THALF,
                            :,
                        ],
                        in_=stg[
                            :, nck * THALF * G:(nck + 1) * THALF * G
                        ].rearrange("p (t g) -> p t g", t=THALF),
                    )
    nc.compile()
    return nc


def _get_nc():
    global _NC_CACHE
    if _NC_CACHE is None:
        _NC_CACHE = _build_nc()
    return _NC_CACHE


def _make_in_maps(x, W1, b1, W2, b2):
    if COMPUTE == "bf16":
        import ml_dtypes
        mmnp = ml_dtypes.bfloat16
    else:
        mmnp = np.float32
    x = np.ascontiguousarray(np.asarray(x, dtype=np.float32))
    W1 = np.asarray(W1, dtype=np.float32)
    b1 = np.asarray(b1, dtype=np.float32)
    W2 = np.asarray(W2, dtype=np.float32)
    b2 = np.asarray(b2, dtype=np.float32)

    kt = _wavelet_kt().reshape(MCHUNK, 128, N).astype(mmnp)
    w1t = np.ascontiguousarray(np.concatenate([W1.T, W1.T], axis=0)).astype(mmnp)
    w2t = np.ascontiguousarray(W2.T).reshape(2, 128, G).astype(mmnp)
    b1r = np.ascontiguousarray(b1.reshape(2, 128, 1))
    b22 = np.ascontiguousarray(
        np.tile(b2.reshape(1, G), (1, 2))
    ).astype(mmnp)
    ones = np.ones((1, 128), dtype=mmnp)

    in_maps = []
    for c in range(NCORES):
        xc = x[c * BPC:(c + 1) * BPC].reshape(BPC, N, TD)
        xc = np.ascontiguousarray(xc.reshape(BPC, MCHUNK, 128, TD).astype(mmnp))
        in_maps.append(
            {"x": xc, "KT": kt, "W1T": w1t, "W2T": w2t, "b1": b1r,
             "b22": b22, "ones": ones}
        )
    return in_maps


def kernel(x, W1, b1, W2, b2):
    nc = _get_nc()
    in_maps = _make_in_maps(x, W1, b1, W2, b2)
    res = run_bass_kernel_spmd(nc, in_maps, list(range(NCORES)))
    out = np.concatenate([res.results[c]["out"] for c in range(NCORES)], axis=0)
    return out


# revision 12
# speedup vs baseline: 5.4087x; 1.0283x over previous
"""Trainium2 Bass kernel for nn_Encoder_inter: coif1 wavelet disentangle along
the node axis (expressed as a dense 512x512 matrix, precomputed on host) followed
by a 2-layer MLP (64->256->256) with ReLU, pointwise over (B, N, T).

Sharding: data-parallel over batch B=32 across 8 NeuronCores (4 batches each);
the small Linear weights and the wavelet matrix are replicated.
"""
import os
import sys

for _p in ("/opt/trn_rl_repo", "/root/.axon_site/_ro/trn_rl_repo"):
    if os.path.isdir(_p) and _p not in sys.path:
        sys.path.insert(0, _p)

from contextlib import ExitStack

import numpy as np

import concourse.bass as bass
import concourse.tile as tile
from concourse import bacc, mybir
from concourse.bass_utils import run_bass_kernel_spmd

F32 = mybir.dt.float32
F32R = mybir.dt.float32r
BF16 = mybir.dt.bfloat16

# compute dtype for tensor-engine operands: "bf16" or "f32r"
COMPUTE = os.environ.get("KERNEL_COMPUTE_DTYPE", "bf16")
MM_DT = BF16 if COMPUTE == "bf16" else F32R

B, N, T, D, H, G = 32, 512, 24, 64, 256, 256
NCORES = 8
BPC = B // NCORES          # batches per core
TD = T * D                 # 1536
NCHUNK = N // 128          # 4
MCHUNK = N // 128          # 4
THALF = T // 2             # 12

# ---------------------------------------------------------------------------
# Host-side wavelet matrix: the whole dwt -> (2*cD) -> idwt chain along the
# node axis is linear, so it is exactly y = K @ x with K (N, N). We build
# K^T = op(eye(N)) in float64 with a numpy port of the reference transform.
# ---------------------------------------------------------------------------
_L = 6
_DEC_LO = np.array(
    [-0.01565572813546454, -0.0727326195128539, 0.38486484686420286,
     0.8525720202122554, 0.3378976624578092, -0.0727326195128539],
    dtype=np.float64,
)
_DEC_HI = np.array(
    [0.0727326195128539, 0.3378976624578092, -0.8525720202122554,
     0.38486484686420286, 0.0727326195128539, -0.01565572813546454],
    dtype=np.float64,
)
_REC_LO = _DEC_LO[::-1].copy()
_REC_HI = _DEC_HI[::-1].copy()


def _dwt_last(x):
    n = x.shape[-1]
    ext = np.concatenate(
        [x[..., : _L - 1][..., ::-1], x, x[..., -(_L - 1):][..., ::-1]], axis=-1
    )
    out = (n + _L - 2) // 2
    cA = sum(_DEC_LO[j] * ext[..., _L - j: _L - j + 2 * out: 2] for j in range(_L))
    cD = sum(_DEC_HI[j] * ext[..., _L - j: _L - j + 2 * out: 2] for j in range(_L))
    return cA, cD


def _idwt_last(cA, cD, n):
    out = cA.shape[-1]
    up_shape = cA.shape[:-1] + (2 * out - 1,)
    upA = np.zeros(up_shape, cA.dtype)
    upA[..., ::2] = cA
    upD = np.zeros(up_shape, cD.dtype)
    upD[..., ::2] = cD
    pad = [(0, 0)] * (cA.ndim - 1) + [(_L - 1, _L - 1)]
    uA = np.pad(upA, pad)
    uD = np.pad(upD, pad)
    return sum(
        _REC_LO[j] * uA[..., 2 * _L - 3 - j: 2 * _L - 3 - j + n]
        + _REC_HI[j] * uD[..., 2 * _L - 3 - j: 2 * _L - 3 - j + n]
        for j in range(_L)
    )


def _wavelet_kt() -> np.ndarray:
    """K^T (m_in, n_out) so that (op(x))[n] = sum_m x[m] * KT[m, n]."""
    eye = np.eye(N, dtype=np.float64)
    cA, cD = _dwt_last(eye)
    kt = _idwt_last(cA, 2.0 * cD, N)
    return kt.astype(np.float32)


# ---------------------------------------------------------------------------
# Device kernel (SPMD, identical program on all 8 cores)
# ---------------------------------------------------------------------------
_NC_CACHE = None


def _build_nc():
    nc = bacc.Bacc("TRN2", target_bir_lowering=False, debug=False, num_devices=NCORES)
    x_d = nc.dram_tensor("x", [BPC, MCHUNK, 128, TD], MM_DT, kind="ExternalInput").ap()
    kt_d = nc.dram_tensor("KT", [MCHUNK, 128, N], MM_DT, kind="ExternalInput").ap()
    w1_d = nc.dram_tensor("W1T", [2 * D, H], MM_DT, kind="ExternalInput").ap()
    w2_d = nc.dram_tensor("W2T", [2, 128, G], MM_DT, kind="ExternalInput").ap()
    b1_d = nc.dram_tensor("b1", [2, 128, 1], F32, kind="ExternalInput").ap()
    b22_d = nc.dram_tensor("b22", [1, 2 * G], MM_DT, kind="ExternalInput").ap()
    ones_d = nc.dram_tensor("ones", [1, 128], MM_DT, kind="ExternalInput").ap()
    out_d = nc.dram_tensor("out", [BPC, N, T, G], MM_DT, kind="ExternalOutput").ap()

    relu = mybir.ActivationFunctionType.Relu

    with tile.TileContext(nc) as tc, ExitStack() as ctx:
        consts = ctx.enter_context(tc.tile_pool(name="consts", bufs=1))
        xpool = ctx.enter_context(tc.tile_pool(name="xp", bufs=2))
        ypool = ctx.enter_context(tc.tile_pool(name="yp", bufs=3))
        hpool = ctx.enter_context(tc.tile_pool(name="hp", bufs=2))
        spool = ctx.enter_context(tc.tile_pool(name="sp", bufs=2))
        py = ctx.enter_context(tc.tile_pool(name="py", bufs=2, space="PSUM"))
        ph = ctx.enter_context(tc.tile_pool(name="ph", bufs=2, space="PSUM"))
        po = ctx.enter_context(tc.tile_pool(name="po", bufs=2, space="PSUM"))

        # --- replicated constants ---
        kt_sb = []
        for mc in range(MCHUNK):
            t_ = consts.tile([128, N], MM_DT, tag=f"kt{mc}", name=f"kt{mc}")
            nc.sync.dma_start(out=t_[:], in_=kt_d[mc])
            kt_sb.append(t_)
        w1_sb = consts.tile([2 * D, H], MM_DT, tag="w1", name="w1")
        nc.sync.dma_start(out=w1_sb[:], in_=w1_d[:])
        w2_sb = []
        for hc in range(2):
            t_ = consts.tile([128, G], MM_DT, tag=f"w2{hc}", name=f"w2{hc}")
            nc.sync.dma_start(out=t_[:], in_=w2_d[hc])
            w2_sb.append(t_)
        b1_sb = []
        for hc in range(2):
            t_ = consts.tile([128, 1], F32, tag=f"b1{hc}", name=f"b1c{hc}")
            nc.sync.dma_start(out=t_[:], in_=b1_d[hc])
            b1_sb.append(t_)
        b22_sb = consts.tile([1, 2 * G], MM_DT, tag="b22", name="b22")
        nc.sync.dma_start(out=b22_sb[:], in_=b22_d[:])
        ones_sb = consts.tile([1, 128], MM_DT, tag="ones", name="ones")
        nc.sync.dma_start(out=ones_sb[:], in_=ones_d[:])

        for b in range(BPC):
            x_sb = []
            for mc in range(MCHUNK):
                t_ = xpool.tile([128, TD], MM_DT, tag=f"x{mc}", name=f"xt{mc}")
                nc.sync.dma_start(out=t_[:], in_=x_d[b, mc])
                x_sb.append(t_)
            for half in range(2):
                stg = spool.tile(
                    [128, NCHUNK * THALF * G], MM_DT, tag="stg", name="stg"
                )
                stg4 = stg[:].rearrange("p (k t g) -> p k t g", k=NCHUNK, t=THALF)
                for tp in range(THALF // 2):
                    t0 = half * THALF + 2 * tp
                    # step 1 (t-pair): psum rows = [t0 d | t1 d], cols = n
                    yps = py.tile([128, N], F32, name="yps")
                    for mc in range(MCHUNK):
                        nc.tensor.matmul(
                            yps[:],
                            lhsT=x_sb[mc][:, t0 * D:(t0 + 2) * D],
                            rhs=kt_sb[mc][:],
                            start=(mc == 0),
                            stop=(mc == MCHUNK - 1),
                        )
                    y_sb = ypool.tile([128, N], MM_DT, tag="yt", name="y_sb")
                    nc.scalar.copy(y_sb[:], yps[:])
                    # step 2: per hc, both t of the pair into one 2-bank psum
                    h1 = []
                    for hc in range(2):
                        hps = ph.tile([128, 2 * N], F32, name="hps")
                        for ti in range(2):
                            nc.tensor.matmul(
                                hps[:, ti * N:(ti + 1) * N],
                                lhsT=w1_sb[
                                    ti * D:(ti + 1) * D, hc * 128:(hc + 1) * 128
                                ],
                                rhs=y_sb[ti * D:(ti + 1) * D, :],
                                start=True,
                                stop=True,
                            )
                        h_sb = hpool.tile(
                            [128, 2 * N], MM_DT, tag=f"h1_{hc}", name=f"h1_{hc}"
                        )
                        nc.scalar.activation(
                            h_sb[:], hps[:], relu, bias=b1_sb[hc][:], scale=1.0
                        )
                        h1.append(h_sb)
                    # step 3: nck-pairs share one psum bank (128, 512)
                    for ti in range(2):
                        tl = 2 * tp + ti
                        for nckp in range(NCHUNK // 2):
                            ops = po.tile([128, 2 * G], F32, name="ops")
                            nc.tensor.matmul(
                                ops[:],
                                lhsT=ones_sb[:],
                                rhs=b22_sb[:],
                                start=True,
                                stop=False,
                                skip_group_check=True,
                            )
                            for sub in range(2):
                                nck = 2 * nckp + sub
                                for hc in range(2):
                                    nc.tensor.matmul(
                                        ops[:, sub * G:(sub + 1) * G],
                                        lhsT=h1[hc][
                                            :,
                                            ti * N + nck * 128:ti * N + (nck + 1) * 128,
                                        ],
                                        rhs=w2_sb[hc][:],
                                        start=False,
                                        stop=(sub == 1 and hc == 1),
                                        skip_group_check=True,
                                    )
                            nc.vector.tensor_scalar_max(
                                stg4[:, 2 * nckp:2 * nckp + 2, tl, :],
                                ops[:].rearrange("p (k g) -> p k g", k=2),
                                0.0,
                            )
                for nck in range(NCHUNK):
                    nc.sync.dma_start(
                        out=out_d[
                            b,
                            nck * 128:(nck + 1) * 128,
                            half * THALF:(half + 1) * THALF,
                            :,
                        ],
                        in_=stg[
                            :, nck * THALF * G:(nck + 1) * THALF * G
                        ].rearrange("p (t g) -> p t g", t=THALF),
                    )
    nc.compile()
    return nc


def _get_nc():
    global _NC_CACHE
    if _NC_CACHE is None:
        _NC_CACHE = _build_nc()
    return _NC_CACHE


def _make_in_maps(x, W1, b1, W2, b2):
    if COMPUTE == "bf16":
        import ml_dtypes
        mmnp = ml_dtypes.bfloat16
    else:
        mmnp = np.float32
    x = np.ascontiguousarray(np.asarray(x, dtype=np.float32))
    W1 = np.asarray(W1, dtype=np.float32)
    b1 = np.asarray(b1, dtype=np.float32)
    W2 = np.asarray(W2, dtype=np.float32)
    b2 = np.asarray(b2, dtype=np.float32)

    kt = _wavelet_kt().reshape(MCHUNK, 128, N).astype(mmnp)
    w1t = np.ascontiguousarray(np.concatenate([W1.T, W1.T], axis=0)).astype(mmnp)
    w2t = np.ascontiguousarray(W2.T).reshape(2, 128, G).astype(mmnp)
    b1r = np.ascontiguousarray(b1.reshape(2, 128, 1))
    b22 = np.ascontiguousarray(
        np.tile(b2.reshape(1, G), (1, 2))
    ).astype(mmnp)
    ones = np.ones((1, 128), dtype=mmnp)

    in_maps = []
    for c in range(NCORES):
        xc = x[c * BPC:(c + 1) * BPC].reshape(BPC, N, TD)
        xc = np.ascontiguousarray(xc.reshape(BPC, MCHUNK, 128, TD).astype(mmnp))
        in_maps.append(
            {"x": xc, "KT": kt, "W1T": w1t, "W2T": w2t, "b1": b1r,
             "b22": b22, "ones": ones}
        )
    return in_maps


def kernel(x, W1, b1, W2, b2):
    nc = _get_nc()
    in_maps = _make_in_maps(x, W1, b1, W2, b2)
    res = run_bass_kernel_spmd(nc, in_maps, list(range(NCORES)))
    out = np.concatenate([res.results[c]["out"] for c in range(NCORES)], axis=0)
    return np.ascontiguousarray(out.astype(np.float32))


# revision 13
# speedup vs baseline: 6.0378x; 1.1163x over previous
"""Trainium2 Bass kernel for nn_Encoder_inter: coif1 wavelet disentangle along
the node axis (expressed as a dense 512x512 matrix, precomputed on host) followed
by a 2-layer MLP (64->256->256) with ReLU, pointwise over (B, N, T).

Sharding: data-parallel over batch B=32 across 8 NeuronCores (4 batches each);
the small Linear weights and the wavelet matrix are replicated.
"""
import os
import sys

for _p in ("/opt/trn_rl_repo", "/root/.axon_site/_ro/trn_rl_repo"):
    if os.path.isdir(_p) and _p not in sys.path:
        sys.path.insert(0, _p)

from contextlib import ExitStack

import numpy as np

import concourse.bass as bass
import concourse.tile as tile
from concourse import bacc, mybir
from concourse.bass_utils import run_bass_kernel_spmd

F32 = mybir.dt.float32
F32R = mybir.dt.float32r
BF16 = mybir.dt.bfloat16

# compute dtype for tensor-engine operands: "bf16" or "f32r"
COMPUTE = os.environ.get("KERNEL_COMPUTE_DTYPE", "bf16")
MM_DT = BF16 if COMPUTE == "bf16" else F32R

B, N, T, D, H, G = 32, 512, 24, 64, 256, 256
NCORES = 8
BPC = B // NCORES          # batches per core
TD = T * D                 # 1536
NCHUNK = N // 128          # 4
MCHUNK = N // 128          # 4
THALF = T // 2             # 12

# ---------------------------------------------------------------------------
# Host-side wavelet matrix: the whole dwt -> (2*cD) -> idwt chain along the
# node axis is linear, so it is exactly y = K @ x with K (N, N). We build
# K^T = op(eye(N)) in float64 with a numpy port of the reference transform.
# ---------------------------------------------------------------------------
_L = 6
_DEC_LO = np.array(
    [-0.01565572813546454, -0.0727326195128539, 0.38486484686420286,
     0.8525720202122554, 0.3378976624578092, -0.0727326195128539],
    dtype=np.float64,
)
_DEC_HI = np.array(
    [0.0727326195128539, 0.3378976624578092, -0.8525720202122554,
     0.38486484686420286, 0.0727326195128539, -0.01565572813546454],
    dtype=np.float64,
)
_REC_LO = _DEC_LO[::-1].copy()
_REC_HI = _DEC_HI[::-1].copy()


def _dwt_last(x):
    n = x.shape[-1]
    ext = np.concatenate(
        [x[..., : _L - 1][..., ::-1], x, x[..., -(_L - 1):][..., ::-1]], axis=-1
    )
    out = (n + _L - 2) // 2
    cA = sum(_DEC_LO[j] * ext[..., _L - j: _L - j + 2 * out: 2] for j in range(_L))
    cD = sum(_DEC_HI[j] * ext[..., _L - j: _L - j + 2 * out: 2] for j in range(_L))
    return cA, cD


def _idwt_last(cA, cD, n):
    out = cA.shape[-1]
    up_shape = cA.shape[:-1] + (2 * out - 1,)
    upA = np.zeros(up_shape, cA.dtype)
    upA[..., ::2] = cA
    upD = np.zeros(up_shape, cD.dtype)
    upD[..., ::2] = cD
    pad = [(0, 0)] * (cA.ndim - 1) + [(_L - 1, _L - 1)]
    uA = np.pad(upA, pad)
    uD = np.pad(upD, pad)
    return sum(
        _REC_LO[j] * uA[..., 2 * _L - 3 - j: 2 * _L - 3 - j + n]
        + _REC_HI[j] * uD[..., 2 * _L - 3 - j: 2 * _L - 3 - j + n]
        for j in range(_L)
    )


def _wavelet_kt() -> np.ndarray:
    """K^T (m_in, n_out) so that (op(x))[n] = sum_m x[m] * KT[m, n]."""
    eye = np.eye(N, dtype=np.float64)
    cA, cD = _dwt_last(eye)
    kt = _idwt_last(cA, 2.0 * cD, N)
    return kt.astype(np.float32)


# ---------------------------------------------------------------------------
# Device kernel (SPMD, identical program on all 8 cores)
# ---------------------------------------------------------------------------
_NC_CACHE = None


def _build_nc():
    nc = bacc.Bacc("TRN2", target_bir_lowering=False, debug=False, num_devices=NCORES)
    x_d = nc.dram_tensor("x", [BPC, MCHUNK, 128, TD], MM_DT, kind="ExternalInput").ap()
    kt_d = nc.dram_tensor("KT", [MCHUNK, 128, N], MM_DT, kind="ExternalInput").ap()
    w1_d = nc.dram_tensor("W1T", [2 * D, H], MM_DT, kind="ExternalInput").ap()
    w2_d = nc.dram_tensor("W2T", [2, 128, G], MM_DT, kind="ExternalInput").ap()
    b1_d = nc.dram_tensor("b1", [2, 128, 1], F32, kind="ExternalInput").ap()
    b22_d = nc.dram_tensor("b22", [128, 2 * G], MM_DT, kind="ExternalInput").ap()
    ones_d = nc.dram_tensor("ones", [128, 128], MM_DT, kind="ExternalInput").ap()
    out_d = nc.dram_tensor("out", [BPC, N, T, G], MM_DT, kind="ExternalOutput").ap()

    relu = mybir.ActivationFunctionType.Relu

    with tile.TileContext(nc) as tc, ExitStack() as ctx:
        consts = ctx.enter_context(tc.tile_pool(name="consts", bufs=1))
        xpool = ctx.enter_context(tc.tile_pool(name="xp", bufs=2))
        ypool = ctx.enter_context(tc.tile_pool(name="yp", bufs=3))
        hpool = ctx.enter_context(tc.tile_pool(name="hp", bufs=2))
        spool = ctx.enter_context(tc.tile_pool(name="sp", bufs=2))
        py = ctx.enter_context(tc.tile_pool(name="py", bufs=2, space="PSUM"))
        ph = ctx.enter_context(tc.tile_pool(name="ph", bufs=2, space="PSUM"))
        po = ctx.enter_context(tc.tile_pool(name="po", bufs=2, space="PSUM"))

        # --- replicated constants ---
        kt_sb = []
        for mc in range(MCHUNK):
            t_ = consts.tile([128, N], MM_DT, tag=f"kt{mc}", name=f"kt{mc}")
            nc.sync.dma_start(out=t_[:], in_=kt_d[mc])
            kt_sb.append(t_)
        w1_sb = consts.tile([2 * D, H], MM_DT, tag="w1", name="w1")
        nc.sync.dma_start(out=w1_sb[:], in_=w1_d[:])
        w2_sb = []
        for hc in range(2):
            t_ = consts.tile([128, G], MM_DT, tag=f"w2{hc}", name=f"w2{hc}")
            nc.sync.dma_start(out=t_[:], in_=w2_d[hc])
            w2_sb.append(t_)
        b1_sb = []
        for hc in range(2):
            t_ = consts.tile([128, 1], F32, tag=f"b1{hc}", name=f"b1c{hc}")
            nc.sync.dma_start(out=t_[:], in_=b1_d[hc])
            b1_sb.append(t_)
        b22_sb = consts.tile([128, 2 * G], MM_DT, tag="b22", name="b22")
        nc.sync.dma_start(out=b22_sb[:], in_=b22_d[:])
        ones_sb = consts.tile([128, 128], MM_DT, tag="ones", name="ones")
        nc.sync.dma_start(out=ones_sb[:], in_=ones_d[:])

        for b in range(BPC):
            x_sb = []
            for mc in range(MCHUNK):
                t_ = xpool.tile([128, TD], MM_DT, tag=f"x{mc}", name=f"xt{mc}")
                nc.sync.dma_start(out=t_[:], in_=x_d[b, mc])
                x_sb.append(t_)
            for half in range(2):
                stg = spool.tile(
                    [128, NCHUNK * THALF * G], MM_DT, tag="stg", name="stg"
                )
                stg4 = stg[:].rearrange("p (k t g) -> p k t g", k=NCHUNK, t=THALF)
                for tp in range(THALF // 2):
                    t0 = half * THALF + 2 * tp
                    # step 1 (t-pair): psum rows = [t0 d | t1 d], cols = n
                    yps = py.tile([128, N], F32, name="yps")
                    for mc in range(MCHUNK):
                        lo, hi = (0, N) if mc == 0 else (128 * mc - 4, min(N, 128 * mc + 132))
                        nc.tensor.matmul(
                            yps[:, lo:hi],
                            lhsT=x_sb[mc][:, t0 * D:(t0 + 2) * D],
                            rhs=kt_sb[mc][:, lo:hi],
                            start=(mc == 0),
                            stop=(mc == MCHUNK - 1),
                            skip_group_check=True,
                        )
                    y_sb = ypool.tile([128, N], MM_DT, tag="yt", name="y_sb")
                    nc.scalar.copy(y_sb[:], yps[:])
                    # step 2: per hc, both t of the pair into one 2-bank psum
                    h1 = []
                    for hc in range(2):
                        hps = ph.tile([128, 2 * N], F32, name="hps")
                        for ti in range(2):
                            nc.tensor.matmul(
                                hps[:, ti * N:(ti + 1) * N],
                                lhsT=w1_sb[
                                    ti * D:(ti + 1) * D, hc * 128:(hc + 1) * 128
                                ],
                                rhs=y_sb[ti * D:(ti + 1) * D, :],
                                start=True,
                                stop=True,
                            )
                        h_sb = hpool.tile(
                            [128, 2 * N], MM_DT, tag=f"h1_{hc}", name=f"h1_{hc}"
                        )
                        nc.scalar.activation(
                            h_sb[:], hps[:], relu, bias=b1_sb[hc][:], scale=1.0
                        )
                        h1.append(h_sb)
                    # step 3: nck-pairs share one psum bank (128, 512)
                    for ti in range(2):
                        tl = 2 * tp + ti
                        for nckp in range(NCHUNK // 2):
                            ops = po.tile([128, 2 * G], F32, name="ops")
                            nc.tensor.matmul(
                                ops[:],
                                lhsT=ones_sb[:],
                                rhs=b22_sb[:],
                                start=True,
                                stop=False,
                                skip_group_check=True,
                            )
                            for sub in range(2):
                                nck = 2 * nckp + sub
                                for hc in range(2):
                                    nc.tensor.matmul(
                                        ops[:, sub * G:(sub + 1) * G],
                                        lhsT=h1[hc][
                                            :,
                                            ti * N + nck * 128:ti * N + (nck + 1) * 128,
                                        ],
                                        rhs=w2_sb[hc][:],
                                        start=False,
                                        stop=(sub == 1 and hc == 1),
                                        skip_group_check=True,
                                    )
                            nc.vector.tensor_scalar_max(
                                stg4[:, 2 * nckp:2 * nckp + 2, tl, :],
                                ops[:].rearrange("p (k g) -> p k g", k=2),
                                0.0,
                            )
                for nck in range(NCHUNK):
                    nc.sync.dma_start(
                        out=out_d[
                            b,
                            nck * 128:(nck + 1) * 128,
                            half * THALF:(half + 1) * THALF,
                            :,
                        ],
                        in_=stg[
                            :, nck * THALF * G:(nck + 1) * THALF * G
                        ].rearrange("p (t g) -> p t g", t=THALF),
                    )
    nc.compile()
    return nc


def _get_nc():
    global _NC_CACHE
    if _NC_CACHE is None:
        _NC_CACHE = _build_nc()
    return _NC_CACHE


def _make_in_maps(x, W1, b1, W2, b2):
    if COMPUTE == "bf16":
        import ml_dtypes
        mmnp = ml_dtypes.bfloat16
    else:
        mmnp = np.float32
    x = np.ascontiguousarray(np.asarray(x, dtype=np.float32))
    W1 = np.asarray(W1, dtype=np.float32)
    b1 = np.asarray(b1, dtype=np.float32)
    W2 = np.asarray(W2, dtype=np.float32)
    b2 = np.asarray(b2, dtype=np.float32)

    kt = _wavelet_kt().reshape(MCHUNK, 128, N).astype(mmnp)
    w1t = np.ascontiguousarray(np.concatenate([W1.T, W1.T], axis=0)).astype(mmnp)
    w2t = np.ascontiguousarray(W2.T).reshape(2, 128, G).astype(mmnp)
    b1r = np.ascontiguousarray(b1.reshape(2, 128, 1))
    b22 = np.ascontiguousarray(
        np.tile((b2 / 128.0).reshape(1, G), (128, 2))
    ).astype(mmnp)
    ones = np.ones((128, 128), dtype=mmnp)

    in_maps = []
    for c in range(NCORES):
        xc = x[c * BPC:(c + 1) * BPC].reshape(BPC, N, TD)
        xc = np.ascontiguousarray(xc.reshape(BPC, MCHUNK, 128, TD).astype(mmnp))
        in_maps.append(
            {"x": xc, "KT": kt, "W1T": w1t, "W2T": w2t, "b1": b1r,
             "b22": b22, "ones": ones}
        )
    return in_maps


def kernel(x, W1, b1, W2, b2):
    nc = _get_nc()
    in_maps = _make_in_maps(x, W1, b1, W2, b2)
    res = run_bass_kernel_spmd(nc, in_maps, list(range(NCORES)))
    out = np.concatenate([res.results[c]["out"] for c in range(NCORES)], axis=0)
    return np.ascontiguousarray(out.astype(np.float32))


# revision 15
# speedup vs baseline: 6.1134x; 1.0125x over previous
"""Trainium2 Bass kernel for nn_Encoder_inter: coif1 wavelet disentangle along
the node axis (expressed as a dense 512x512 matrix, precomputed on host) followed
by a 2-layer MLP (64->256->256) with ReLU, pointwise over (B, N, T).

Sharding: data-parallel over batch B=32 across 8 NeuronCores (4 batches each);
the small Linear weights and the wavelet matrix are replicated.
"""
import os
import sys

for _p in ("/opt/trn_rl_repo", "/root/.axon_site/_ro/trn_rl_repo"):
    if os.path.isdir(_p) and _p not in sys.path:
        sys.path.insert(0, _p)

from contextlib import ExitStack

import numpy as np

import concourse.bass as bass
import concourse.tile as tile
from concourse import bacc, mybir
from concourse.bass_utils import run_bass_kernel_spmd

F32 = mybir.dt.float32
F32R = mybir.dt.float32r
BF16 = mybir.dt.bfloat16

# compute dtype for tensor-engine operands: "bf16" or "f32r"
COMPUTE = os.environ.get("KERNEL_COMPUTE_DTYPE", "bf16")
MM_DT = BF16 if COMPUTE == "bf16" else F32R

B, N, T, D, H, G = 32, 512, 24, 64, 256, 256
NCORES = 8
BPC = B // NCORES          # batches per core
TD = T * D                 # 1536
NCHUNK = N // 128          # 4
MCHUNK = N // 128          # 4
THALF = T // 2             # 12

# ---------------------------------------------------------------------------
# Host-side wavelet matrix: the whole dwt -> (2*cD) -> idwt chain along the
# node axis is linear, so it is exactly y = K @ x with K (N, N). We build
# K^T = op(eye(N)) in float64 with a numpy port of the reference transform.
# ---------------------------------------------------------------------------
_L = 6
_DEC_LO = np.array(
    [-0.01565572813546454, -0.0727326195128539, 0.38486484686420286,
     0.8525720202122554, 0.3378976624578092, -0.0727326195128539],
    dtype=np.float64,
)
_DEC_HI = np.array(
    [0.0727326195128539, 0.3378976624578092, -0.8525720202122554,
     0.38486484686420286, 0.0727326195128539, -0.01565572813546454],
    dtype=np.float64,
)
_REC_LO = _DEC_LO[::-1].copy()
_REC_HI = _DEC_HI[::-1].copy()


def _dwt_last(x):
    n = x.shape[-1]
    ext = np.concatenate(
        [x[..., : _L - 1][..., ::-1], x, x[..., -(_L - 1):][..., ::-1]], axis=-1
    )
    out = (n + _L - 2) // 2
    cA = sum(_DEC_LO[j] * ext[..., _L - j: _L - j + 2 * out: 2] for j in range(_L))
    cD = sum(_DEC_HI[j] * ext[..., _L - j: _L - j + 2 * out: 2] for j in range(_L))
    return cA, cD


def _idwt_last(cA, cD, n):
    out = cA.shape[-1]
    up_shape = cA.shape[:-1] + (2 * out - 1,)
    upA = np.zeros(up_shape, cA.dtype)
    upA[..., ::2] = cA
    upD = np.zeros(up_shape, cD.dtype)
    upD[..., ::2] = cD
    pad = [(0, 0)] * (cA.ndim - 1) + [(_L - 1, _L - 1)]
    uA = np.pad(upA, pad)
    uD = np.pad(upD, pad)
    return sum(
        _REC_LO[j] * uA[..., 2 * _L - 3 - j: 2 * _L - 3 - j + n]
        + _REC_HI[j] * uD[..., 2 * _L - 3 - j: 2 * _L - 3 - j + n]
        for j in range(_L)
    )


def _wavelet_kt() -> np.ndarray:
    """K^T (m_in, n_out) so that (op(x))[n] = sum_m x[m] * KT[m, n]."""
    eye = np.eye(N, dtype=np.float64)
    cA, cD = _dwt_last(eye)
    kt = _idwt_last(cA, 2.0 * cD, N)
    return kt.astype(np.float32)


# ---------------------------------------------------------------------------
# Device kernel (SPMD, identical program on all 8 cores)
# ---------------------------------------------------------------------------
_NC_CACHE = None


def _build_nc():
    nc = bacc.Bacc("TRN2", target_bir_lowering=False, debug=False, num_devices=NCORES)
    x_d = nc.dram_tensor("x", [BPC, MCHUNK, 128, TD], MM_DT, kind="ExternalInput").ap()
    kt_d = nc.dram_tensor("KT", [MCHUNK, 128, N], MM_DT, kind="ExternalInput").ap()
    w1_d = nc.dram_tensor("W1T", [2 * D, H], MM_DT, kind="ExternalInput").ap()
    w2_d = nc.dram_tensor("W2T", [2, 128, G], MM_DT, kind="ExternalInput").ap()
    b1_d = nc.dram_tensor("b1", [2, 128, 1], F32, kind="ExternalInput").ap()
    b22_d = nc.dram_tensor("b22", [128, 2 * G], MM_DT, kind="ExternalInput").ap()
    ones_d = nc.dram_tensor("ones", [128, 128], MM_DT, kind="ExternalInput").ap()
    out_d = nc.dram_tensor("out", [BPC, N, T, G], MM_DT, kind="ExternalOutput").ap()

    relu = mybir.ActivationFunctionType.Relu

    with tile.TileContext(nc) as tc, ExitStack() as ctx:
        consts = ctx.enter_context(tc.tile_pool(name="consts", bufs=1))
        xpool = ctx.enter_context(tc.tile_pool(name="xp", bufs=2))
        ypool = ctx.enter_context(tc.tile_pool(name="yp", bufs=3))
        hpool = ctx.enter_context(tc.tile_pool(name="hp", bufs=2))
        spool = ctx.enter_context(tc.tile_pool(name="sp", bufs=2))
        py = ctx.enter_context(tc.tile_pool(name="py", bufs=2, space="PSUM"))
        ph = ctx.enter_context(tc.tile_pool(name="ph", bufs=2, space="PSUM"))
        po = ctx.enter_context(tc.tile_pool(name="po", bufs=2, space="PSUM"))

        # --- replicated constants ---
        kt_sb = []
        for mc in range(MCHUNK):
            t_ = consts.tile([128, N], MM_DT, tag=f"kt{mc}", name=f"kt{mc}")
            nc.sync.dma_start(out=t_[:], in_=kt_d[mc])
            kt_sb.append(t_)
        w1_sb = consts.tile([2 * D, H], MM_DT, tag="w1", name="w1")
        nc.sync.dma_start(out=w1_sb[:], in_=w1_d[:])
        w2_sb = []
        for hc in range(2):
            t_ = consts.tile([128, G], MM_DT, tag=f"w2{hc}", name=f"w2{hc}")
            nc.sync.dma_start(out=t_[:], in_=w2_d[hc])
            w2_sb.append(t_)
        b1_sb = []
        for hc in range(2):
            t_ = consts.tile([128, 1], F32, tag=f"b1{hc}", name=f"b1c{hc}")
            nc.sync.dma_start(out=t_[:], in_=b1_d[hc])
            b1_sb.append(t_)
        b22_sb = consts.tile([128, 2 * G], MM_DT, tag="b22", name="b22")
        nc.sync.dma_start(out=b22_sb[:], in_=b22_d[:])
        ones_sb = consts.tile([128, 128], MM_DT, tag="ones", name="ones")
        nc.sync.dma_start(out=ones_sb[:], in_=ones_d[:])

        for b in range(BPC):
            x_sb = []
            for mc in range(MCHUNK):
                t_ = xpool.tile([128, TD], MM_DT, tag=f"x{mc}", name=f"xt{mc}")
                nc.sync.dma_start(out=t_[:], in_=x_d[b, mc])
                x_sb.append(t_)
            for half in range(2):
                stg = spool.tile(
                    [128, NCHUNK * THALF * G], MM_DT, tag="stg", name="stg"
                )
                stg4 = stg[:].rearrange("p (k t g) -> p k t g", k=NCHUNK, t=THALF)
                for tp in range(THALF // 2):
                    t0 = half * THALF + 2 * tp
                    # step 1 (t-pair): psum rows = [t0 d | t1 d], cols = n
                    yps = py.tile([128, N], F32, name="yps")
                    for mc in range(MCHUNK):
                        if mc == 0:
                            windows = [(0, 256, True), (256, N, False)]
                        else:
                            windows = [
                                (128 * mc - 4, min(N, 128 * mc + 132), False)
                            ]
                        for lo, hi, st in windows:
                            nc.tensor.matmul(
                                yps[:, lo:hi],
                                lhsT=x_sb[mc][:, t0 * D:(t0 + 2) * D],
                                rhs=kt_sb[mc][:, lo:hi],
                                start=st,
                                stop=(mc == MCHUNK - 1),
                                skip_group_check=True,
                            )
                    y_sb = ypool.tile([128, N], MM_DT, tag="yt", name="y_sb")
                    nc.scalar.copy(y_sb[:], yps[:])
                    # step 2: per hc, both t of the pair into one 2-bank psum
                    h1 = []
                    for hc in range(2):
                        hps = ph.tile([128, 2 * N], F32, name="hps")
                        for ti in range(2):
                            for qq in range(2):
                                nc.tensor.matmul(
                                    hps[:, ti * N + qq * 256:ti * N + (qq + 1) * 256],
                                    lhsT=w1_sb[
                                        ti * D:(ti + 1) * D, hc * 128:(hc + 1) * 128
                                    ],
                                    rhs=y_sb[ti * D:(ti + 1) * D, qq * 256:(qq + 1) * 256],
                                    start=(qq == 0),
                                    stop=(qq == 1),
                                    skip_group_check=True,
                                )
                        h_sb = hpool.tile(
                            [128, 2 * N], MM_DT, tag=f"h1_{hc}", name=f"h1_{hc}"
                        )
                        nc.scalar.activation(
                            h_sb[:], hps[:], relu, bias=b1_sb[hc][:], scale=1.0
                        )
                        h1.append(h_sb)
                    # step 3: nck-pairs share one psum bank (128, 512)
                    for ti in range(2):
                        tl = 2 * tp + ti
                        for nckp in range(NCHUNK // 2):
                            ops = po.tile([128, 2 * G], F32, name="ops")
                            for qq in range(2):
                                nc.tensor.matmul(
                                    ops[:, qq * G:(qq + 1) * G],
                                    lhsT=ones_sb[:],
                                    rhs=b22_sb[:, qq * G:(qq + 1) * G],
                                    start=(qq == 0),
                                    stop=False,
                                    skip_group_check=True,
                                )
                            for sub in range(2):
                                nck = 2 * nckp + sub
                                for hc in range(2):
                                    nc.tensor.matmul(
                                        ops[:, sub * G:(sub + 1) * G],
                                        lhsT=h1[hc][
                                            :,
                                            ti * N + nck * 128:ti * N + (nck + 1) * 128,
                                        ],
                                        rhs=w2_sb[hc][:],
                                        start=False,
                                        stop=(sub == 1 and hc == 1),
                                        skip_group_check=True,
                                    )
                            nc.vector.tensor_scalar_max(
                                stg4[:, 2 * nckp:2 * nckp + 2, tl, :],
                                ops[:].rearrange("p (k g) -> p k g", k=2),
                                0.0,
                            )
                for nck in range(NCHUNK):
                    nc.sync.dma_start(
                        out=out_d[
                            b,
                            nck * 128:(nck + 1) * 128,
                            half * THALF:(half + 1) * THALF,
                            :,
                        ],
                        in_=stg[
                            :, nck * THALF * G:(nck + 1) * THALF * G
                        ].rearrange("p (t g) -> p t g", t=THALF),
                    )
    nc.compile()
    return nc


def _get_nc():
    global _NC_CACHE
    if _NC_CACHE is None:
        _NC_CACHE = _build_nc()
    return _NC_CACHE


def _make_in_maps(x, W1, b1, W2, b2):
    if COMPUTE == "bf16":
        import ml_dtypes
        mmnp = ml_dtypes.bfloat16
    else:
        mmnp = np.float32
    x = np.ascontiguousarray(np.asarray(x, dtype=np.float32))
    W1 = np.asarray(W1, dtype=np.float32)
    b1 = np.asarray(b1, dtype=np.float32)
    W2 = np.asarray(W2, dtype=np.float32)
    b2 = np.asarray(b2, dtype=np.float32)

    kt = _wavelet_kt().reshape(MCHUNK, 128, N).astype(mmnp)
    w1t = np.ascontiguousarray(np.concatenate([W1.T, W1.T], axis=0)).astype(mmnp)
    w2t = np.ascontiguousarray(W2.T).reshape(2, 128, G).astype(mmnp)
    b1r = np.ascontiguousarray(b1.reshape(2, 128, 1))
    b22 = np.ascontiguousarray(
        np.tile((b2 / 128.0).reshape(1, G), (128, 2))
    ).astype(mmnp)
    ones = np.ones((128, 128), dtype=mmnp)

    in_maps = []
    for c in range(NCORES):
        xc = x[c * BPC:(c + 1) * BPC].reshape(BPC, N, TD)
        xc = np.ascontiguousarray(xc.reshape(BPC, MCHUNK, 128, TD).astype(mmnp))
        in_maps.append(
            {"x": xc, "KT": kt, "W1T": w1t, "W2T": w2t, "b1": b1r,
             "b22": b22, "ones": ones}
        )
    return in_maps


def kernel(x, W1, b1, W2, b2):
    nc = _get_nc()
    in_maps = _make_in_maps(x, W1, b1, W2, b2)
    res = run_bass_kernel_spmd(nc, in_maps, list(range(NCORES)))
    out = np.concatenate([res.results[c]["out"] for c in range(NCORES)], axis=0)
    return np.ascontiguousarray(out.astype(np.float32))


# revision 16
# speedup vs baseline: 6.2746x; 1.0264x over previous
"""Trainium2 Bass kernel for nn_Encoder_inter: coif1 wavelet disentangle along
the node axis (expressed as a dense 512x512 matrix, precomputed on host) followed
by a 2-layer MLP (64->256->256) with ReLU, pointwise over (B, N, T).

Sharding: data-parallel over batch B=32 across 8 NeuronCores (4 batches each);
the small Linear weights and the wavelet matrix are replicated.
"""
import os
import sys

for _p in ("/opt/trn_rl_repo", "/root/.axon_site/_ro/trn_rl_repo"):
    if os.path.isdir(_p) and _p not in sys.path:
        sys.path.insert(0, _p)

from contextlib import ExitStack

import numpy as np

import concourse.bass as bass
import concourse.tile as tile
from concourse import bacc, mybir
from concourse.bass_utils import run_bass_kernel_spmd

F32 = mybir.dt.float32
F32R = mybir.dt.float32r
BF16 = mybir.dt.bfloat16

# compute dtype for tensor-engine operands: "bf16" or "f32r"
COMPUTE = os.environ.get("KERNEL_COMPUTE_DTYPE", "bf16")
MM_DT = BF16 if COMPUTE == "bf16" else F32R

B, N, T, D, H, G = 32, 512, 24, 64, 256, 256
NCORES = 8
BPC = B // NCORES          # batches per core
TD = T * D                 # 1536
NCHUNK = N // 128          # 4
MCHUNK = N // 128          # 4
THALF = T // 2             # 12

# ---------------------------------------------------------------------------
# Host-side wavelet matrix: the whole dwt -> (2*cD) -> idwt chain along the
# node axis is linear, so it is exactly y = K @ x with K (N, N). We build
# K^T = op(eye(N)) in float64 with a numpy port of the reference transform.
# ---------------------------------------------------------------------------
_L = 6
_DEC_LO = np.array(
    [-0.01565572813546454, -0.0727326195128539, 0.38486484686420286,
     0.8525720202122554, 0.3378976624578092, -0.0727326195128539],
    dtype=np.float64,
)
_DEC_HI = np.array(
    [0.0727326195128539, 0.3378976624578092, -0.8525720202122554,
     0.38486484686420286, 0.0727326195128539, -0.01565572813546454],
    dtype=np.float64,
)
_REC_LO = _DEC_LO[::-1].copy()
_REC_HI = _DEC_HI[::-1].copy()


def _dwt_last(x):
    n = x.shape[-1]
    ext = np.concatenate(
        [x[..., : _L - 1][..., ::-1], x, x[..., -(_L - 1):][..., ::-1]], axis=-1
    )
    out = (n + _L - 2) // 2
    cA = sum(_DEC_LO[j] * ext[..., _L - j: _L - j + 2 * out: 2] for j in range(_L))
    cD = sum(_DEC_HI[j] * ext[..., _L - j: _L - j + 2 * out: 2] for j in range(_L))
    return cA, cD


def _idwt_last(cA, cD, n):
    out = cA.shape[-1]
    up_shape = cA.shape[:-1] + (2 * out - 1,)
    upA = np.zeros(up_shape, cA.dtype)
    upA[..., ::2] = cA
    upD = np.zeros(up_shape, cD.dtype)
    upD[..., ::2] = cD
    pad = [(0, 0)] * (cA.ndim - 1) + [(_L - 1, _L - 1)]
    uA = np.pad(upA, pad)
    uD = np.pad(upD, pad)
    return sum(
        _REC_LO[j] * uA[..., 2 * _L - 3 - j: 2 * _L - 3 - j + n]
        + _REC_HI[j] * uD[..., 2 * _L - 3 - j: 2 * _L - 3 - j + n]
        for j in range(_L)
    )


def _wavelet_kt() -> np.ndarray:
    """K^T (m_in, n_out) so that (op(x))[n] = sum_m x[m] * KT[m, n]."""
    eye = np.eye(N, dtype=np.float64)
    cA, cD = _dwt_last(eye)
    kt = _idwt_last(cA, 2.0 * cD, N)
    return kt.astype(np.float32)


# ---------------------------------------------------------------------------
# Device kernel (SPMD, identical program on all 8 cores)
# ---------------------------------------------------------------------------
_NC_CACHE = None


def _build_nc():
    nc = bacc.Bacc("TRN2", target_bir_lowering=False, debug=False, num_devices=NCORES)
    x_d = nc.dram_tensor("x", [BPC, MCHUNK, 128, TD], MM_DT, kind="ExternalInput").ap()
    kt_d = nc.dram_tensor("KT", [MCHUNK, 128, N], MM_DT, kind="ExternalInput").ap()
    w1_d = nc.dram_tensor("W1T", [2 * D, H], MM_DT, kind="ExternalInput").ap()
    w2_d = nc.dram_tensor("W2T", [2, 128, G], MM_DT, kind="ExternalInput").ap()
    b1_d = nc.dram_tensor("b1", [2, 128, 1], F32, kind="ExternalInput").ap()
    b22_d = nc.dram_tensor("b22", [128, 2 * G], MM_DT, kind="ExternalInput").ap()
    ones_d = nc.dram_tensor("ones", [128, 128], MM_DT, kind="ExternalInput").ap()
    out_d = nc.dram_tensor("out", [BPC, N, T, G], MM_DT, kind="ExternalOutput").ap()

    relu = mybir.ActivationFunctionType.Relu

    with tile.TileContext(nc) as tc, ExitStack() as ctx:
        consts = ctx.enter_context(tc.tile_pool(name="consts", bufs=1))
        xpool = ctx.enter_context(tc.tile_pool(name="xp", bufs=2))
        ypool = ctx.enter_context(tc.tile_pool(name="yp", bufs=3))
        hpool = ctx.enter_context(tc.tile_pool(name="hp", bufs=2))
        spool = ctx.enter_context(tc.tile_pool(name="sp", bufs=2))
        py = ctx.enter_context(tc.tile_pool(name="py", bufs=2, space="PSUM"))
        ph = ctx.enter_context(tc.tile_pool(name="ph", bufs=2, space="PSUM"))
        po = ctx.enter_context(tc.tile_pool(name="po", bufs=2, space="PSUM"))

        # --- replicated constants ---
        kt_sb = []
        for mc in range(MCHUNK):
            t_ = consts.tile([128, N], MM_DT, tag=f"kt{mc}", name=f"kt{mc}")
            nc.sync.dma_start(out=t_[:], in_=kt_d[mc])
            kt_sb.append(t_)
        w1_sb = consts.tile([2 * D, H], MM_DT, tag="w1", name="w1")
        nc.sync.dma_start(out=w1_sb[:], in_=w1_d[:])
        w2_sb = []
        for hc in range(2):
            t_ = consts.tile([128, G], MM_DT, tag=f"w2{hc}", name=f"w2{hc}")
            nc.sync.dma_start(out=t_[:], in_=w2_d[hc])
            w2_sb.append(t_)
        b1_sb = []
        for hc in range(2):
            t_ = consts.tile([128, 1], F32, tag=f"b1{hc}", name=f"b1c{hc}")
            nc.sync.dma_start(out=t_[:], in_=b1_d[hc])
            b1_sb.append(t_)
        b22_sb = consts.tile([128, 2 * G], MM_DT, tag="b22", name="b22")
        nc.sync.dma_start(out=b22_sb[:], in_=b22_d[:])
        ones_sb = consts.tile([128, 128], MM_DT, tag="ones", name="ones")
        nc.sync.dma_start(out=ones_sb[:], in_=ones_d[:])

        for b in range(BPC):
            x_sb = []
            for mc in range(MCHUNK):
                t_ = xpool.tile([128, TD], MM_DT, tag=f"x{mc}", name=f"xt{mc}")
                for hf in range(2):
                    nc.sync.dma_start(
                        out=t_[:, hf * (TD // 2):(hf + 1) * (TD // 2)],
                        in_=x_d[b, mc][:, hf * (TD // 2):(hf + 1) * (TD // 2)],
                    )
                x_sb.append(t_)
            for half in range(2):
                stgs = [
                    spool.tile(
                        [128, NCHUNK * (THALF // 2) * G], MM_DT, tag=f"stg{q}",
                        name=f"stg{q}",
                    )
                    for q in range(2)
                ]
                stg4s = [
                    s[:].rearrange("p (k t g) -> p k t g", k=NCHUNK, t=THALF // 2)
                    for s in stgs
                ]
                for tp in range(THALF // 2):
                    t0 = half * THALF + 2 * tp
                    # step 1 (t-pair): psum rows = [t0 d | t1 d], cols = n
                    yps = py.tile([128, N], F32, name="yps")
                    for mc in range(MCHUNK):
                        if mc == 0:
                            windows = [(0, 256, True), (256, N, False)]
                        else:
                            windows = [
                                (128 * mc - 4, min(N, 128 * mc + 132), False)
                            ]
                        for lo, hi, st in windows:
                            nc.tensor.matmul(
                                yps[:, lo:hi],
                                lhsT=x_sb[mc][:, t0 * D:(t0 + 2) * D],
                                rhs=kt_sb[mc][:, lo:hi],
                                start=st,
                                stop=(mc == MCHUNK - 1),
                                skip_group_check=True,
                            )
                    y_sb = ypool.tile([128, N], MM_DT, tag="yt", name="y_sb")
                    nc.scalar.copy(y_sb[:], yps[:])
                    # step 2: per hc, both t of the pair into one 2-bank psum
                    h1 = []
                    for hc in range(2):
                        hps = ph.tile([128, 2 * N], F32, name="hps")
                        for qq in range(2):
                            for ti in range(2):
                                nc.tensor.matmul(
                                    hps[:, ti * N + qq * 256:ti * N + (qq + 1) * 256],
                                    lhsT=w1_sb[
                                        ti * D:(ti + 1) * D, hc * 128:(hc + 1) * 128
                                    ],
                                    rhs=y_sb[ti * D:(ti + 1) * D, qq * 256:(qq + 1) * 256],
                                    start=(qq == 0),
                                    stop=(qq == 1),
                                    skip_group_check=True,
                                )
                        h_sb = hpool.tile(
                            [128, 2 * N], MM_DT, tag=f"h1_{hc}", name=f"h1_{hc}"
                        )
                        nc.scalar.activation(
                            h_sb[:], hps[:], relu, bias=b1_sb[hc][:], scale=1.0
                        )
                        h1.append(h_sb)
                    # step 3: nck-pairs share one psum bank (128, 512)
                    for ti in range(2):
                        tl = 2 * tp + ti
                        for nckp in range(NCHUNK // 2):
                            ops = po.tile([128, 2 * G], F32, name="ops")
                            for qq in range(2):
                                nc.tensor.matmul(
                                    ops[:, qq * G:(qq + 1) * G],
                                    lhsT=ones_sb[:],
                                    rhs=b22_sb[:, qq * G:(qq + 1) * G],
                                    start=(qq == 0),
                                    stop=False,
                                    skip_group_check=True,
                                )
                            for sub in range(2):
                                nck = 2 * nckp + sub
                                for hc in range(2):
                                    nc.tensor.matmul(
                                        ops[:, sub * G:(sub + 1) * G],
                                        lhsT=h1[hc][
                                            :,
                                            ti * N + nck * 128:ti * N + (nck + 1) * 128,
                                        ],
                                        rhs=w2_sb[hc][:],
                                        start=False,
                                        stop=(sub == 1 and hc == 1),
                                        skip_group_check=True,
                                    )
                            nc.vector.tensor_scalar_max(
                                stg4s[tl // 6][:, 2 * nckp:2 * nckp + 2, tl % 6, :],
                                ops[:].rearrange("p (k g) -> p k g", k=2),
                                0.0,
                            )
                for q in range(2):
                    tq = THALF // 2
                    for nck in range(NCHUNK):
                        nc.sync.dma_start(
                            out=out_d[
                                b,
                                nck * 128:(nck + 1) * 128,
                                half * THALF + q * tq:half * THALF + (q + 1) * tq,
                                :,
                            ],
                            in_=stgs[q][
                                :, nck * tq * G:(nck + 1) * tq * G
                            ].rearrange("p (t g) -> p t g", t=tq),
                        )
    nc.compile()
    return nc


def _get_nc():
    global _NC_CACHE
    if _NC_CACHE is None:
        _NC_CACHE = _build_nc()
    return _NC_CACHE


def _make_in_maps(x, W1, b1, W2, b2):
    if COMPUTE == "bf16":
        import ml_dtypes
        mmnp = ml_dtypes.bfloat16
    else:
        mmnp = np.float32
    x = np.ascontiguousarray(np.asarray(x, dtype=np.float32))
    W1 = np.asarray(W1, dtype=np.float32)
    b1 = np.asarray(b1, dtype=np.float32)
    W2 = np.asarray(W2, dtype=np.float32)
    b2 = np.asarray(b2, dtype=np.float32)

    kt = _wavelet_kt().reshape(MCHUNK, 128, N).astype(mmnp)
    w1t = np.ascontiguousarray(np.concatenate([W1.T, W1.T], axis=0)).astype(mmnp)
    w2t = np.ascontiguousarray(W2.T).reshape(2, 128, G).astype(mmnp)
    b1r = np.ascontiguousarray(b1.reshape(2, 128, 1))
    b22 = np.ascontiguousarray(
        np.tile((b2 / 128.0).reshape(1, G), (128, 2))
    ).astype(mmnp)
    ones = np.ones((128, 128), dtype=mmnp)

    in_maps = []
    for c in range(NCORES):
        xc = x[c * BPC:(c + 1) * BPC].reshape(BPC, N, TD)
        xc = np.ascontiguousarray(xc.reshape(BPC, MCHUNK, 128, TD).astype(mmnp))
        in_maps.append(
            {"x": xc, "KT": kt, "W1T": w1t, "W2T": w2t, "b1": b1r,
             "b22": b22, "ones": ones}
        )
    return in_maps


def kernel(x, W1, b1, W2, b2):
    nc = _get_nc()
    in_maps = _make_in_maps(x, W1, b1, W2, b2)
    res = run_bass_kernel_spmd(nc, in_maps, list(range(NCORES)))
    out = np.concatenate([res.results[c]["out"] for c in range(NCORES)], axis=0)
    return np.ascontiguousarray(out.astype(np.float32))


# revision 17
# speedup vs baseline: 6.2991x; 1.0039x over previous
"""Trainium2 Bass kernel for nn_Encoder_inter: coif1 wavelet disentangle along
the node axis (expressed as a dense 512x512 matrix, precomputed on host) followed
by a 2-layer MLP (64->256->256) with ReLU, pointwise over (B, N, T).

Sharding: data-parallel over batch B=32 across 8 NeuronCores (4 batches each);
the small Linear weights and the wavelet matrix are replicated.
"""
import os
import sys

for _p in ("/opt/trn_rl_repo", "/root/.axon_site/_ro/trn_rl_repo"):
    if os.path.isdir(_p) and _p not in sys.path:
        sys.path.insert(0, _p)

from contextlib import ExitStack

import numpy as np

import concourse.bass as bass
import concourse.tile as tile
from concourse import bacc, mybir
from concourse.bass_utils import run_bass_kernel_spmd

F32 = mybir.dt.float32
F32R = mybir.dt.float32r
BF16 = mybir.dt.bfloat16

# compute dtype for tensor-engine operands: "bf16" or "f32r"
COMPUTE = os.environ.get("KERNEL_COMPUTE_DTYPE", "bf16")
MM_DT = BF16 if COMPUTE == "bf16" else F32R

B, N, T, D, H, G = 32, 512, 24, 64, 256, 256
NCORES = 8
BPC = B // NCORES          # batches per core
TD = T * D                 # 1536
NCHUNK = N // 128          # 4
MCHUNK = N // 128          # 4
THALF = T // 2             # 12

# ---------------------------------------------------------------------------
# Host-side wavelet matrix: the whole dwt -> (2*cD) -> idwt chain along the
# node axis is linear, so it is exactly y = K @ x with K (N, N). We build
# K^T = op(eye(N)) in float64 with a numpy port of the reference transform.
# ---------------------------------------------------------------------------
_L = 6
_DEC_LO = np.array(
    [-0.01565572813546454, -0.0727326195128539, 0.38486484686420286,
     0.8525720202122554, 0.3378976624578092, -0.0727326195128539],
    dtype=np.float64,
)
_DEC_HI = np.array(
    [0.0727326195128539, 0.3378976624578092, -0.8525720202122554,
     0.38486484686420286, 0.0727326195128539, -0.01565572813546454],
    dtype=np.float64,
)
_REC_LO = _DEC_LO[::-1].copy()
_REC_HI = _DEC_HI[::-1].copy()


def _dwt_last(x):
    n = x.shape[-1]
    ext = np.concatenate(
        [x[..., : _L - 1][..., ::-1], x, x[..., -(_L - 1):][..., ::-1]], axis=-1
    )
    out = (n + _L - 2) // 2
    cA = sum(_DEC_LO[j] * ext[..., _L - j: _L - j + 2 * out: 2] for j in range(_L))
    cD = sum(_DEC_HI[j] * ext[..., _L - j: _L - j + 2 * out: 2] for j in range(_L))
    return cA, cD


def _idwt_last(cA, cD, n):
    out = cA.shape[-1]
    up_shape = cA.shape[:-1] + (2 * out - 1,)
    upA = np.zeros(up_shape, cA.dtype)
    upA[..., ::2] = cA
    upD = np.zeros(up_shape, cD.dtype)
    upD[..., ::2] = cD
    pad = [(0, 0)] * (cA.ndim - 1) + [(_L - 1, _L - 1)]
    uA = np.pad(upA, pad)
    uD = np.pad(upD, pad)
    return sum(
        _REC_LO[j] * uA[..., 2 * _L - 3 - j: 2 * _L - 3 - j + n]
        + _REC_HI[j] * uD[..., 2 * _L - 3 - j: 2 * _L - 3 - j + n]
        for j in range(_L)
    )


def _wavelet_kt() -> np.ndarray:
    """K^T (m_in, n_out) so that (op(x))[n] = sum_m x[m] * KT[m, n]."""
    eye = np.eye(N, dtype=np.float64)
    cA, cD = _dwt_last(eye)
    kt = _idwt_last(cA, 2.0 * cD, N)
    return kt.astype(np.float32)


# ---------------------------------------------------------------------------
# Device kernel (SPMD, identical program on all 8 cores)
# ---------------------------------------------------------------------------
_NC_CACHE = None


def _build_nc():
    nc = bacc.Bacc("TRN2", target_bir_lowering=False, debug=False, num_devices=NCORES)
    x_d = nc.dram_tensor("x", [BPC, MCHUNK, 128, TD], MM_DT, kind="ExternalInput").ap()
    kt_d = nc.dram_tensor("KT", [MCHUNK, 128, N], MM_DT, kind="ExternalInput").ap()
    w1_d = nc.dram_tensor("W1T", [2 * D, H], MM_DT, kind="ExternalInput").ap()
    w2_d = nc.dram_tensor("W2T", [2, 128, G], MM_DT, kind="ExternalInput").ap()
    b1_d = nc.dram_tensor("b1", [2, 128, 1], F32, kind="ExternalInput").ap()
    b22_d = nc.dram_tensor("b22", [128, 2 * G], MM_DT, kind="ExternalInput").ap()
    ones_d = nc.dram_tensor("ones", [128, 128], MM_DT, kind="ExternalInput").ap()
    out_d = nc.dram_tensor("out", [BPC, N, T, G], MM_DT, kind="ExternalOutput").ap()

    relu = mybir.ActivationFunctionType.Relu

    with tile.TileContext(nc) as tc, ExitStack() as ctx:
        consts = ctx.enter_context(tc.tile_pool(name="consts", bufs=1))
        xpool = ctx.enter_context(tc.tile_pool(name="xp", bufs=2))
        ypool = ctx.enter_context(tc.tile_pool(name="yp", bufs=3))
        hpool = ctx.enter_context(tc.tile_pool(name="hp", bufs=2))
        spool = ctx.enter_context(tc.tile_pool(name="sp", bufs=2))
        py = ctx.enter_context(tc.tile_pool(name="py", bufs=2, space="PSUM"))
        ph = ctx.enter_context(tc.tile_pool(name="ph", bufs=2, space="PSUM"))
        po = ctx.enter_context(tc.tile_pool(name="po", bufs=2, space="PSUM"))

        # --- replicated constants ---
        kt_sb = []
        for mc in range(MCHUNK):
            t_ = consts.tile([128, N], MM_DT, tag=f"kt{mc}", name=f"kt{mc}")
            nc.sync.dma_start(out=t_[:], in_=kt_d[mc])
            kt_sb.append(t_)
        w1_sb = consts.tile([2 * D, H], MM_DT, tag="w1", name="w1")
        nc.sync.dma_start(out=w1_sb[:], in_=w1_d[:])
        w2_sb = []
        for hc in range(2):
            t_ = consts.tile([128, G], MM_DT, tag=f"w2{hc}", name=f"w2{hc}")
            nc.sync.dma_start(out=t_[:], in_=w2_d[hc])
            w2_sb.append(t_)
        b1_sb = []
        for hc in range(2):
            t_ = consts.tile([128, 1], F32, tag=f"b1{hc}", name=f"b1c{hc}")
            nc.sync.dma_start(out=t_[:], in_=b1_d[hc])
            b1_sb.append(t_)
        b22_sb = consts.tile([128, 2 * G], MM_DT, tag="b22", name="b22")
        nc.sync.dma_start(out=b22_sb[:], in_=b22_d[:])
        ones_sb = consts.tile([128, 128], MM_DT, tag="ones", name="ones")
        nc.sync.dma_start(out=ones_sb[:], in_=ones_d[:])

        for b in range(BPC):
            x_sb = [
                xpool.tile([128, TD], MM_DT, tag=f"x{mc}", name=f"xt{mc}")
                for mc in range(MCHUNK)
            ]
            for hf in range(2):
                for mc in range(MCHUNK):
                    nc.sync.dma_start(
                        out=x_sb[mc][:, hf * (TD // 2):(hf + 1) * (TD // 2)],
                        in_=x_d[b, mc][:, hf * (TD // 2):(hf + 1) * (TD // 2)],
                    )
            for half in range(2):
                stgs = [
                    spool.tile(
                        [128, NCHUNK * (THALF // 2) * G], MM_DT, tag=f"stg{q}",
                        name=f"stg{q}",
                    )
                    for q in range(2)
                ]
                stg4s = [
                    s[:].rearrange("p (k t g) -> p k t g", k=NCHUNK, t=THALF // 2)
                    for s in stgs
                ]
                for tp in range(THALF // 2):
                    t0 = half * THALF + 2 * tp
                    # step 1 (t-pair): psum rows = [t0 d | t1 d], cols = n
                    yps = py.tile([128, N], F32, name="yps")
                    for mc in range(MCHUNK):
                        if mc == 0:
                            windows = [(0, 256, True), (256, N, False)]
                        else:
                            windows = [
                                (128 * mc - 4, min(N, 128 * mc + 132), False)
                            ]
                        for lo, hi, st in windows:
                            nc.tensor.matmul(
                                yps[:, lo:hi],
                                lhsT=x_sb[mc][:, t0 * D:(t0 + 2) * D],
                                rhs=kt_sb[mc][:, lo:hi],
                                start=st,
                                stop=(mc == MCHUNK - 1),
                                skip_group_check=True,
                            )
                    y_sb = ypool.tile([128, N], MM_DT, tag="yt", name="y_sb")
                    nc.scalar.copy(y_sb[:], yps[:])
                    # step 2: per hc, both t of the pair into one 2-bank psum
                    h1 = []
                    for hc in range(2):
                        hps = ph.tile([128, 2 * N], F32, name="hps")
                        for qq in range(2):
                            for ti in range(2):
                                nc.tensor.matmul(
                                    hps[:, ti * N + qq * 256:ti * N + (qq + 1) * 256],
                                    lhsT=w1_sb[
                                        ti * D:(ti + 1) * D, hc * 128:(hc + 1) * 128
                                    ],
                                    rhs=y_sb[ti * D:(ti + 1) * D, qq * 256:(qq + 1) * 256],
                                    start=(qq == 0),
                                    stop=(qq == 1),
                                    skip_group_check=True,
                                )
                        h_sb = hpool.tile(
                            [128, 2 * N], MM_DT, tag=f"h1_{hc}", name=f"h1_{hc}"
                        )
                        nc.scalar.activation(
                            h_sb[:], hps[:], relu, bias=b1_sb[hc][:], scale=1.0
                        )
                        h1.append(h_sb)
                    # step 3: nck-pairs share one psum bank (128, 512)
                    for ti in range(2):
                        tl = 2 * tp + ti
                        for nckp in range(NCHUNK // 2):
                            ops = po.tile([128, 2 * G], F32, name="ops")
                            for qq in range(2):
                                nc.tensor.matmul(
                                    ops[:, qq * G:(qq + 1) * G],
                                    lhsT=ones_sb[:],
                                    rhs=b22_sb[:, qq * G:(qq + 1) * G],
                                    start=(qq == 0),
                                    stop=False,
                                    skip_group_check=True,
                                )
                            for sub in range(2):
                                nck = 2 * nckp + sub
                                for hc in range(2):
                                    nc.tensor.matmul(
                                        ops[:, sub * G:(sub + 1) * G],
                                        lhsT=h1[hc][
                                            :,
                                            ti * N + nck * 128:ti * N + (nck + 1) * 128,
                                        ],
                                        rhs=w2_sb[hc][:],
                                        start=False,
                                        stop=(sub == 1 and hc == 1),
                                        skip_group_check=True,
                                    )
                            nc.vector.tensor_scalar_max(
                                stg4s[tl // 6][:, 2 * nckp:2 * nckp + 2, tl % 6, :],
                                ops[:].rearrange("p (k g) -> p k g", k=2),
                                0.0,
                            )
                for q in range(2):
                    tq = THALF // 2
                    for nck in range(NCHUNK):
                        nc.sync.dma_start(
                            out=out_d[
                                b,
                                nck * 128:(nck + 1) * 128,
                                half * THALF + q * tq:half * THALF + (q + 1) * tq,
                                :,
                            ],
                            in_=stgs[q][
                                :, nck * tq * G:(nck + 1) * tq * G
                            ].rearrange("p (t g) -> p t g", t=tq),
                        )
    nc.compile()
    return nc


def _get_nc():
    global _NC_CACHE
    if _NC_CACHE is None:
        _NC_CACHE = _build_nc()
    return _NC_CACHE


def _make_in_maps(x, W1, b1, W2, b2):
    if COMPUTE == "bf16":
        import ml_dtypes
        mmnp = ml_dtypes.bfloat16
    else:
        mmnp = np.float32
    x = np.ascontiguousarray(np.asarray(x, dtype=np.float32))
    W1 = np.asarray(W1, dtype=np.float32)
    b1 = np.asarray(b1, dtype=np.float32)
    W2 = np.asarray(W2, dtype=np.float32)
    b2 = np.asarray(b2, dtype=np.float32)

    kt = _wavelet_kt().reshape(MCHUNK, 128, N).astype(mmnp)
    w1t = np.ascontiguousarray(np.concatenate([W1.T, W1.T], axis=0)).astype(mmnp)
    w2t = np.ascontiguousarray(W2.T).reshape(2, 128, G).astype(mmnp)
    b1r = np.ascontiguousarray(b1.reshape(2, 128, 1))
    b22 = np.ascontiguousarray(
        np.tile((b2 / 128.0).reshape(1, G), (128, 2))
    ).astype(mmnp)
    ones = np.ones((128, 128), dtype=mmnp)

    in_maps = []
    for c in range(NCORES):
        xc = x[c * BPC:(c + 1) * BPC].reshape(BPC, N, TD)
        xc = np.ascontiguousarray(xc.reshape(BPC, MCHUNK, 128, TD).astype(mmnp))
        in_maps.append(
            {"x": xc, "KT": kt, "W1T": w1t, "W2T": w2t, "b1": b1r,
             "b22": b22, "ones": ones}
        )
    return in_maps


def kernel(x, W1, b1, W2, b2):
    nc = _get_nc()
    in_maps = _make_in_maps(x, W1, b1, W2, b2)
    res = run_bass_kernel_spmd(nc, in_maps, list(range(NCORES)))
    out = np.concatenate([res.results[c]["out"] for c in range(NCORES)], axis=0)
    return np.ascontiguousarray(out.astype(np.float32))
